# revision 1
# baseline (speedup 1.0000x reference)
"""Trainium2 Bass kernel for a LeakyReLU RNN.

Model (B=128, S=512, I=256, H=1024, O=256):
    xproj = lrelu(x @ Wi.T + bi)                          # [B,S,H]
    h_t   = lrelu(concat(xproj_t, h_{t-1}) @ Wh.T + bh)   # recurrence over S
    out   = h_S @ Wo.T + bo                               # [B,O]

Strategy: data-parallel over batch (16 rows/core on 8 cores). Split
Wh = [Wh1 | Wh2]; U = xproj @ Wh1.T + bh is precomputed as big GEMMs,
the sequential part is h_t = lrelu(U_t + h_{t-1} @ Wh2.T) with the
hidden state as the (16-wide) stationary operand and Wh2.T streamed as
the moving operand (fp32r, 1 cycle/column).
"""

from contextlib import ExitStack

import numpy as np

import concourse.bacc as bacc
import concourse.tile as tile
from concourse import mybir
from concourse.bass_utils import run_bass_kernel_spmd

B, S, I, H, O = 128, 512, 256, 1024, 256
NCORES = 8
BL = B // NCORES          # batch rows per core = 16
TOK = BL * S              # tokens per core = 8192
NBLK = TOK // 512         # 512-token blocks in phase 1 = 16
RING_STEPS = 8            # recurrence steps per U ring DMA
ALPHA = 0.01

F32 = mybir.dt.float32
F32R = mybir.dt.float32r
LRELU = mybir.ActivationFunctionType.Lrelu

_CACHED = None


def _build(S=S, NBLK=NBLK):
    TOK = BL * S
    nc = bacc.Bacc("TRN2", target_bir_lowering=False, debug=False,
                   num_devices=NCORES)

    xt_d = nc.dram_tensor("xt", [I, TOK], F32, kind="ExternalInput")
    wit_d = nc.dram_tensor("wit", [I, H], F32, kind="ExternalInput")
    wh1t_d = nc.dram_tensor("wh1t", [H, H], F32, kind="ExternalInput")
    wh2t_d = nc.dram_tensor("wh2t", [H, H], F32, kind="ExternalInput")
    wot_d = nc.dram_tensor("wot", [H, O], F32, kind="ExternalInput")
    bi_d = nc.dram_tensor("bi", [128, H // 128], F32, kind="ExternalInput")
    bh_d = nc.dram_tensor("bh", [1, H], F32, kind="ExternalInput")
    bo_d = nc.dram_tensor("bo", [1, O], F32, kind="ExternalInput")
    ident_d = nc.dram_tensor("ident", [16, 16], F32, kind="ExternalInput")
    eye_d = nc.dram_tensor("eye128", [128, 128], F32, kind="ExternalInput")
    ones_d = nc.dram_tensor("ones", [1, 128], F32, kind="ExternalInput")
    y_d = nc.dram_tensor("y", [BL, O], F32, kind="ExternalOutput")
    u_dram = nc.dram_tensor("udram", [TOK, H], F32R)

    with tile.TileContext(nc) as tc, ExitStack() as ctx:
        wpool = ctx.enter_context(tc.tile_pool(name="weights", bufs=1))
        xtpool = ctx.enter_context(tc.tile_pool(name="xt", bufs=3))
        apool = ctx.enter_context(tc.tile_pool(name="atiles", bufs=2))
        upool = ctx.enter_context(tc.tile_pool(name="usb", bufs=4))
        ringpool = ctx.enter_context(tc.tile_pool(name="uring", bufs=4))
        hpool = ctx.enter_context(tc.tile_pool(name="hbuf", bufs=2))
        opool = ctx.enter_context(tc.tile_pool(name="osb", bufs=1))
        ps1ctx = ExitStack()
        psA = ps1ctx.enter_context(tc.tile_pool(name="psA", bufs=2, space="PSUM"))
        psU = ps1ctx.enter_context(tc.tile_pool(name="psU", bufs=4, space="PSUM"))

        # ---- resident weights (gpsimd DMA casts f32 -> rounded f32r) ----
        def wload(src, shape, tag, dt=F32R):
            t = wpool.tile(shape, dt, tag=tag, name=tag)
            nc.gpsimd.dma_start(t[:], src)
            return t

        wit = [wload(wit_d.ap()[128 * k:128 * (k + 1), :], [128, H], f"wit{k}")
               for k in range(2)]
        wh1t = [wload(wh1t_d.ap()[128 * k:128 * (k + 1), :], [128, H], f"wh1t{k}")
                for k in range(8)]
        wh2t = [wload(wh2t_d.ap()[128 * k:128 * (k + 1), :], [128, H], f"wh2t{k}")
                for k in range(8)]
        wot = [wload(wot_d.ap()[128 * k:128 * (k + 1), :], [128, O], f"wot{k}")
               for k in range(8)]
        eye = wload(eye_d.ap(), [128, 128], "eye")
        bh2 = wload(bh_d.ap(), [1, H], "bh2")
        bo2 = wload(bo_d.ap(), [1, O], "bo2")
        identf = wload(ident_d.ap(), [16, 16], "identf", dt=F32)
        ones = wload(ones_d.ap(), [1, 128], "ones")
        bi = wpool.tile([128, H // 128], F32, tag="bi", name="bi")
        nc.sync.dma_start(bi[:], bi_d.ap())

        # ---- phase 1: A_T = lrelu(WiT.T @ Xt + bi); U = A @ Wh1.T + bh ----
        for blk in range(NBLK):
            c0 = 512 * blk
            xt = [xtpool.tile([128, 512], F32R, tag=f"xt{k}", name=f"xt{k}_{blk}") for k in range(2)]
            for k in range(2):
                nc.gpsimd.dma_start(
                    xt[k][:], xt_d.ap()[128 * k:128 * (k + 1), c0:c0 + 512])
            a = []
            for m in range(8):
                pa = psA.tile([128, 512], F32, tag="psA", name=f"psA_{blk}_{m}")
                nc.tensor.matmul(pa[:], wit[0][:, 128 * m:128 * (m + 1)],
                                 xt[0][:], start=True, stop=False)
                nc.tensor.matmul(pa[:], wit[1][:, 128 * m:128 * (m + 1)],
                                 xt[1][:], start=False, stop=True)
                am = apool.tile([128, 512], F32R, tag=f"a{m}", name=f"a{m}_{blk}")
                nc.scalar.activation(am[:], pa[:], LRELU,
                                     bias=bi[:, m:m + 1], scale=1.0, alpha=ALPHA)
                a.append(am)
            for q in range(4):
                pu = [psU.tile([128, 512], F32, tag="psU", name=f"psU_{blk}_{q}_{n}")
                      for n in range(2)]
                for n in range(2):
                    nc.tensor.matmul(pu[n][:], ones[0:1, 0:128],
                                     bh2[0:1, 512 * n:512 * (n + 1)],
                                     start=True, stop=False)
                for k in range(8):
                    for n in range(2):
                        nc.tensor.matmul(
                            pu[n][:], a[k][:, 128 * q:128 * (q + 1)],
                            wh1t[k][:, 512 * n:512 * (n + 1)],
                            start=False, stop=(k == 7))
                for n in range(2):
                    usb = upool.tile([128, 512], F32R, tag="usb", name=f"usb_{blk}_{q}_{n}")
                    nc.vector.tensor_copy(usb[:], pu[n][:])
                    nc.scalar.dma_start(
                        u_dram.ap()[c0 + 128 * q:c0 + 128 * (q + 1),
                                    512 * n:512 * (n + 1)], usb[:])

        # ---- phase 2: recurrence ----
        ps1ctx.close()
        psR = ctx.enter_context(tc.tile_pool(name="psR", bufs=4, space="PSUM"))
        psT = ctx.enter_context(tc.tile_pool(name="psT", bufs=2, space="PSUM"))
        # hT chunks: 8 tiles [128, 16] (h state transposed), f32r
        hT = []
        for j in range(8):
            t = hpool.tile([128, 16], F32R, tag=f"hT{j}", name=f"hT{j}_init")
            nc.gpsimd.memset(t[:].bitcast(F32), 0.0)
            hT.append(t)

        ring = None
        ps_next = None

        def emit_id_mms(t):
            g = t % RING_STEPS
            sel = eye[:, 16 * g:16 * (g + 1)]
            ps0 = psR.tile([16, 512], F32, tag="psR", name=f"psR0_{t}")
            ps1 = psR.tile([16, 512], F32, tag="psR", name=f"psR1_{t}")
            nc.tensor.matmul(ps0[:], sel, ring[:, 0:512],
                             start=True, stop=False)
            nc.tensor.matmul(ps1[:], sel, ring[:, 512:1024],
                             start=True, stop=False)
            return ps0, ps1

        def load_ring(t):
            ring_new = ringpool.tile([RING_STEPS * BL, H], F32R, tag="ring", name=f"ring_{t}")
            r0 = t * BL
            nc.sync.dma_start(ring_new[:], u_dram.ap()[r0:r0 + RING_STEPS * BL, :])
            return ring_new

        for t in range(S):
            if t == 0:
                ring = load_ring(0)
                ps0, ps1 = emit_id_mms(0)
            else:
                ps0, ps1 = ps_next
            for k in range(8):
                nc.tensor.matmul(ps0[:], hT[k][:], wh2t[k][:, 0:512],
                                 start=False, stop=(k == 7))
            for k in range(8):
                nc.tensor.matmul(ps1[:], hT[k][:], wh2t[k][:, 512:1024],
                                 start=False, stop=(k == 7))
            if t + 1 < S:
                nxt = t + 1
                if nxt % RING_STEPS == 0:
                    ring = load_ring(nxt)
                ps_next = emit_id_mms(nxt)
            hT_new = []
            for c in range(4):
                ps = ps0 if c < 2 else ps1
                off = (c % 2) * 256
                hn = hpool.tile([16, 256], F32R, tag=f"hn{c}", name=f"hn{c}_{t}")
                nc.scalar.activation(hn[:], ps[:, off:off + 256], LRELU,
                                     bias=0.0, scale=1.0, alpha=ALPHA)
                for u in range(2):
                    j = 2 * c + u
                    pt = psT.tile([128, 16], F32R, tag="psT", name=f"psT{j}_{t}")
                    nc.tensor.transpose(pt[:], hn[:, 128 * u:128 * (u + 1)],
                                        eye[0:16, 0:16])
                    ht = hpool.tile([128, 16], F32R, tag=f"hT{j}",
                                    name=f"hT{j}_{t}")
                    nc.vector.tensor_copy(ht[:], pt[:])
                    hT_new.append(ht)
            hT = hT_new

        # ---- phase 3: out = h_S @ Wo.T + bo ----
        po = psT.tile([16, O], F32, tag="psO", name="psO", bufs=1)
        nc.tensor.matmul(po[:], ones[0:1, 0:16], bo2[0:1, :],
                         start=True, stop=False)
        for k in range(8):
            nc.tensor.matmul(po[:], hT[k][:], wot[k][:],
                             start=False, stop=(k == 7))
        osb = opool.tile([16, O], F32, tag="osb", name="osb")
        nc.vector.tensor_copy(osb[:], po[:])
        nc.sync.dma_start(y_d.ap(), osb[:])

    nc.compile()
    return nc


def _prep_inputs(x, Wi, bi, Wh, bh, Wo, bo):
    shared = {
        "wit": np.ascontiguousarray(Wi.T),
        "wh1t": np.ascontiguousarray(Wh[:, :H].T),
        "wh2t": np.ascontiguousarray(Wh[:, H:].T),
        "wot": np.ascontiguousarray(Wo.T),
        "bi": np.ascontiguousarray(bi.reshape(H // 128, 128).T),
        "bh": bh.reshape(1, H).copy(),
        "bo": bo.reshape(1, O).copy(),
        "ident": np.eye(16, dtype=np.float32),
        "eye128": np.eye(128, dtype=np.float32),
        "ones": np.ones((1, 128), np.float32),
    }
    in_maps = []
    for c in range(NCORES):
        xc = x[BL * c:BL * (c + 1)]            # [16, S, I]
        xt = np.ascontiguousarray(
            xc.transpose(2, 1, 0).reshape(I, TOK))  # [I, S*16] col = t*16+b
        m = dict(shared)
        m["xt"] = xt
        in_maps.append(m)
    return in_maps


def kernel(x, Wi, bi, Wh, bh, Wo, bo, _trace=False):
    global _CACHED
    x = np.asarray(x, dtype=np.float32)
    if _CACHED is None:
        _CACHED = _build()
    nc = _CACHED
    in_maps = _prep_inputs(np.asarray(x, np.float32), np.asarray(Wi, np.float32),
                           np.asarray(bi, np.float32), np.asarray(Wh, np.float32),
                           np.asarray(bh, np.float32), np.asarray(Wo, np.float32),
                           np.asarray(bo, np.float32))
    res = run_bass_kernel_spmd(nc, in_maps, list(range(NCORES)), trace=_trace)
    out = np.concatenate([res.results[c]["y"] for c in range(NCORES)], axis=0)
    if _trace:
        return out, res
    return out



# revision 15
# speedup vs baseline: 2.4484x; 2.4484x over previous
"""Trainium2 Bass kernel for a LeakyReLU RNN.

Model (B=128, S=512, I=256, H=1024, O=256):
    xproj = lrelu(x @ Wi.T + bi)                          # [B,S,H]
    h_t   = lrelu(concat(xproj_t, h_{t-1}) @ Wh.T + bh)   # recurrence over S
    out   = h_S @ Wo.T + bo                               # [B,O]

Strategy: data-parallel over batch (16 rows/core on 8 cores). Split
Wh = [Wh1 | Wh2]; U = xproj @ Wh1.T + bh is precomputed blockwise into
SBUF, the sequential part is h_t = lrelu(U_t + h_{t-1} @ Wh2.T).

The recurrence matmuls run in bf16 in 128x32 column-tiled PE mode:
three concurrent tiles (TRN2 forbids matmul dst partition 96), each
with its own moving-operand stream, so the per-step Wh2 stream is
~3072 cycles instead of 8192. Tile j covers output features
[384j, min(384j+384, 1024)) at PSUM partitions [32j, 32j+16). Phase-1
GEMM work for block b+2 is interleaved between the recurrence steps of
window b as PE filler, hiding phase 1 and keeping the PE p-state high.
bf16 is safe here: products accumulate in fp32 PSUM, U is injected
from an fp32-accumulated GEMM, and the recurrence is contractive
(||Wh2|| ~ 0.8), so the per-step ~4e-3 quantization error settles
around 1e-2, within the 2e-2 gate.
"""

from contextlib import ExitStack

import ml_dtypes
import numpy as np

import concourse.bacc as bacc
import concourse.tile as tile
from concourse import mybir
from concourse.bass_utils import run_bass_kernel_spmd

B, S, I, H, O = 128, 512, 256, 1024, 256
NCORES = 8
BL = B // NCORES          # batch rows per core = 16
TOK = BL * S              # tokens per core = 8192
ALPHA = 0.01
NSPLIT = (0, 384, 768, 1024)   # 3-way output-feature split for col tiles

F32 = mybir.dt.float32
F32R = mybir.dt.float32r
BF16 = mybir.dt.bfloat16
LRELU = mybir.ActivationFunctionType.Lrelu

_CACHED = None


def _build(S=S):
    TOK = BL * S
    NBLK = TOK // 512                 # 512-token (32-step) phase-1 blocks
    nc = bacc.Bacc("TRN2", target_bir_lowering=False, debug=False,
                   num_devices=NCORES)

    xt_d = nc.dram_tensor("xt", [I, TOK], F32, kind="ExternalInput")
    wit_d = nc.dram_tensor("wit", [I, H], F32, kind="ExternalInput")
    wh1t_d = nc.dram_tensor("wh1t", [H, H], F32, kind="ExternalInput")
    wh2b_d = nc.dram_tensor("wh2b", [H, H], BF16, kind="ExternalInput")
    wob_d = nc.dram_tensor("wob", [H, O], BF16, kind="ExternalInput")
    bi_d = nc.dram_tensor("bi", [128, H // 128], F32, kind="ExternalInput")
    bhrep_d = nc.dram_tensor("bhrep", [128, H], F32, kind="ExternalInput")
    borep_d = nc.dram_tensor("borep", [128, O], F32, kind="ExternalInput")
    eyeb_d = nc.dram_tensor("eyeb", [128, 128], BF16, kind="ExternalInput")
    y_d = nc.dram_tensor("y", [BL, O], F32, kind="ExternalOutput")

    with tile.TileContext(nc) as tc, ExitStack() as ctx:
        wpool = ctx.enter_context(tc.tile_pool(name="weights", bufs=1))
        xtpool = ctx.enter_context(tc.tile_pool(name="xt", bufs=2))
        apool = ctx.enter_context(tc.tile_pool(name="atiles", bufs=2))
        upool = ctx.enter_context(tc.tile_pool(name="usb", bufs=2))
        hnpool = ctx.enter_context(tc.tile_pool(name="hn", bufs=2))
        htpool = ctx.enter_context(tc.tile_pool(name="ht", bufs=2))
        opool = ctx.enter_context(tc.tile_pool(name="osb", bufs=1))
        psA = ctx.enter_context(tc.tile_pool(name="psA", bufs=2, space="PSUM"))
        psU = ctx.enter_context(tc.tile_pool(name="psU", bufs=1, space="PSUM"))
        psR = ctx.enter_context(tc.tile_pool(name="psR", bufs=2, space="PSUM"))
        psT = ctx.enter_context(tc.tile_pool(name="psT", bufs=2, space="PSUM"))

        # ---- resident weights ----
        def wload(src, shape, tag, dt=F32R, eng=None):
            t = wpool.tile(shape, dt, tag=tag, name=tag)
            (eng or nc.gpsimd).dma_start(t[:], src)
            return t

        wit = [wload(wit_d.ap()[128 * k:128 * (k + 1), :], [128, H], f"wit{k}")
               for k in range(2)]
        wh1t = [wload(wh1t_d.ap()[128 * k:128 * (k + 1), :], [128, H], f"wh1t{k}")
                for k in range(8)]
        wh2b = [wload(wh2b_d.ap()[128 * k:128 * (k + 1), :], [128, H],
                      f"wh2b{k}", dt=BF16, eng=nc.sync)
                for k in range(8)]
        wob = [wload(wob_d.ap()[128 * k:128 * (k + 1), :], [128, O],
                     f"wob{k}", dt=BF16, eng=nc.sync)
               for k in range(8)]
        eyeb = wload(eyeb_d.ap(), [128, 128], "eyeb", dt=BF16, eng=nc.sync)
        bhrep = wload(bhrep_d.ap(), [128, H], "bhrep", dt=F32, eng=nc.sync)
        borep = wload(borep_d.ap(), [128, O], "borep", dt=F32, eng=nc.sync)
        bi = wpool.tile([128, H // 128], F32, tag="bi", name="bi")
        nc.sync.dma_start(bi[:], bi_d.ap())

        # ---- phase 1 (emitted as filler chunks between recurrence steps)
        # Per 512-token block: A_T = lrelu(WiT.T @ Xt + bi)  (feature-major)
        # then U_g = A_g @ Wh1.T + bh for 4 groups of 128 tokens
        # (token-major [128 tok, 1024] bf16), kept in SBUF for the recurrence.
        usb = {}   # (blk % 2, g) -> SBUF tile [128, H]

        def phase1_chunks(blk, t_base):
            """Return a FIFO of (gate, closure); each closure emits
            ~200-450ns of engine work. A closure may only be emitted at
            absolute step >= gate: the usb evacuations overwrite (pool-wise)
            the buffer the current window's injects still read, so they are
            gated past the last consumer; the psU accumulations for the next
            group must stay behind the previous group's evacuation."""
            c0 = 512 * blk
            chunks = []
            xt = []
            a = []

            def dma_chunk():
                for k in range(2):
                    t = xtpool.tile([128, 512], F32R, tag=f"xt{k}",
                                    name=f"xt{k}_{blk}")
                    nc.gpsimd.dma_start(
                        t[:], xt_d.ap()[128 * k:128 * (k + 1), c0:c0 + 512])
                    xt.append(t)
            chunks.append((0, dma_chunk))

            def a_chunk(m):
                pa = psA.tile([128, 512], F32, tag="psA", name=f"psA_{blk}_{m}")
                nc.tensor.matmul(pa[:], wit[0][:, 128 * m:128 * (m + 1)],
                                 xt[0][:], start=True, stop=False)
                nc.tensor.matmul(pa[:], wit[1][:, 128 * m:128 * (m + 1)],
                                 xt[1][:], start=False, stop=True)
                am = apool.tile([128, 512], F32R, tag=f"a{m}", name=f"a{m}_{blk}")
                nc.scalar.activation(am[:], pa[:], LRELU,
                                     bias=bi[:, m:m + 1], scale=1.0, alpha=ALPHA)
                a.append(am)
            for m in range(8):
                chunks.append((0, lambda m=m: a_chunk(m)))

            # U for group g accumulates into psU [128 tok, 1024]
            # (each matmul dst must stay within one 2KB PSUM bank -> n-halves)
            pu = {}

            def u_chunk(g, k):
                if k == 0:
                    pu[g] = psU.tile([128, H], F32, tag="psU",
                                     name=f"psU_{blk}_{g}")
                for n in range(2):
                    nc.tensor.matmul(pu[g][:, 512 * n:512 * (n + 1)],
                                     a[k][:, 128 * g:128 * (g + 1)],
                                     wh1t[k][:, 512 * n:512 * (n + 1)],
                                     start=(k == 0), stop=(k == 7))

            def u_evac(g, q):
                # evacuate in 256-col slices so the DVE never blocks the
                # recurrence's critical hT copy for long; bf16 cast here
                if q == 0:
                    usb[(blk, g)] = upool.tile([128, H], BF16,
                                               tag=f"usb{g}",
                                               name=f"usb{g}_{blk}")
                sl = slice(256 * q, 256 * (q + 1))
                nc.vector.tensor_add(usb[(blk, g)][:, sl], pu[g][:, sl],
                                     bhrep[:, sl])

            for g in range(4):
                # psU has a single buffer: group g's matmuls may only be
                # emitted once group g-1's evacuation is fully emitted, which
                # the FIFO order plus the evac gates below guarantee.
                for k in range(8):
                    chunks.append((0, lambda g=g, k=k: u_chunk(g, k)))
                for q in range(4):
                    # the usb{g} buffer this evac overwrites is read by the
                    # current window's g-group injects (local steps 8g..8g+7)
                    chunks.append((t_base + 8 * g + 7,
                                   lambda g=g, q=q: u_evac(g, q)))
            return chunks

        # ---- recurrence state ----
        ht = htpool.tile([128, 128], BF16, tag="hT", name="hT_init")
        nc.gpsimd.memset(ht[:].bitcast(F32), 0.0)

        def step(t):
            """One recurrence step: 3 col-tiles of bf16 matmuls + transposes.

            Col tile j covers features [NSPLIT[j], NSPLIT[j+1]) and writes
            PSUM partitions [32j, 32j+16)."""
            nonlocal ht
            blk, g, s = t // 32, (t % 32) // 8, t % 8
            ut = usb[(blk, g)]
            ps = psR.tile([128, 384], F32, tag="psR", name=f"psR_{t}")
            sel = eyeb[:, 16 * s:16 * (s + 1)]
            # inject U_t (start) then accumulate h @ Wh2T, 3 col-tiles
            for j in range(3):
                w = NSPLIT[j + 1] - NSPLIT[j]
                nc.tensor.matmul(ps[32 * j:32 * j + 16, 0:w], sel,
                                 ut[:, NSPLIT[j]:NSPLIT[j + 1]],
                                 start=True, stop=False,
                                 tile_position=(0, 32 * j))
            for k in range(8):
                hk = ht[:, 16 * k:16 * (k + 1)]
                for j in range(3):
                    w = NSPLIT[j + 1] - NSPLIT[j]
                    nc.tensor.matmul(ps[32 * j:32 * j + 16, 0:w], hk,
                                     wh2b[k][:, NSPLIT[j]:NSPLIT[j + 1]],
                                     start=False, stop=(k == 7),
                                     tile_position=(0, 32 * j))
            # lrelu over all three quarters in one shot (junk lanes harmless)
            hn = hnpool.tile([128, 384], BF16, tag="hn", name=f"hn_{t}")
            nc.scalar.activation(hn[:], ps[:], LRELU,
                                 bias=0.0, scale=1.0, alpha=ALPHA)
            # transpose back to feature-major stationary for the next step
            pt = psT.tile([128, 128], BF16, tag="psT", name=f"psT_{t}")
            for k in range(8):
                j = (128 * k) // 384          # source col tile
                c = 128 * k - 384 * j         # col offset within tile j
                nc.tensor.transpose(
                    pt[:, 16 * k:16 * (k + 1)],
                    hn[32 * j:32 * j + 16, c:c + 128],
                    eyeb[32 * j:32 * j + 16, 32 * j:32 * j + 16],
                    tile_position=(32 * j, 0))
            ht_new = htpool.tile([128, 128], BF16, tag="hT", name=f"hT_{t}")
            nc.vector.tensor_copy(ht_new[:], pt[:])
            ht = ht_new

        # ---- schedule: prologue blocks 0,1 then steps with filler ----
        for _, c in phase1_chunks(0, 0):
            c()
        if NBLK > 1:
            for _, c in phase1_chunks(1, 0):
                c()
        queue = []
        for t in range(S):
            if t % 32 == 0:
                nblk = t // 32 + 2
                if nblk < NBLK:
                    queue.extend(phase1_chunks(nblk, t))
            step(t)
            # emit filler chunks to keep the PE busy during the act/copy gap
            budget = 3
            while queue and budget > 0 and queue[0][0] <= t:
                queue.pop(0)[1]()
                budget -= 1

        # ---- phase 3: out = h_S @ Wo.T + bo ----
        po = psR.tile([128, 384], F32, tag="psR", name="psO")
        for k in range(8):
            nc.tensor.matmul(po[0:16, 0:O], ht[:, 16 * k:16 * (k + 1)],
                             wob[k][:], start=(k == 0), stop=(k == 7))
        osb = opool.tile([16, O], F32, tag="osb", name="osb")
        nc.vector.tensor_add(osb[:], po[0:16, 0:O], borep[0:16, :])
        nc.sync.dma_start(y_d.ap(), osb[:])

    nc.compile()
    return nc


def _prep_inputs(x, Wi, bi, Wh, bh, Wo, bo):
    bf = ml_dtypes.bfloat16
    shared = {
        "wit": np.ascontiguousarray(Wi.T),
        "wh1t": np.ascontiguousarray(Wh[:, :H].T),
        "wh2b": np.ascontiguousarray(Wh[:, H:].T).astype(bf),
        "wob": np.ascontiguousarray(Wo.T).astype(bf),
        "bi": np.ascontiguousarray(bi.reshape(H // 128, 128).T),
        "bhrep": np.ascontiguousarray(np.broadcast_to(bh.reshape(1, H),
                                                      (128, H))),
        "borep": np.ascontiguousarray(np.broadcast_to(bo.reshape(1, O),
                                                      (128, O))),
        "eyeb": np.eye(128, dtype=np.float32).astype(bf),
    }
    in_maps = []
    for c in range(NCORES):
        xc = x[BL * c:BL * (c + 1)]            # [16, S, I]
        xt = np.ascontiguousarray(
            xc.transpose(2, 1, 0).reshape(I, xc.shape[1] * BL))
        m = dict(shared)
        m["xt"] = xt
        in_maps.append(m)
    return in_maps


def kernel(x, Wi, bi, Wh, bh, Wo, bo, _trace=False):
    global _CACHED
    x = np.asarray(x, dtype=np.float32)
    if _CACHED is None:
        _CACHED = _build()
    nc = _CACHED
    in_maps = _prep_inputs(np.asarray(x, np.float32), np.asarray(Wi, np.float32),
                           np.asarray(bi, np.float32), np.asarray(Wh, np.float32),
                           np.asarray(bh, np.float32), np.asarray(Wo, np.float32),
                           np.asarray(bo, np.float32))
    res = run_bass_kernel_spmd(nc, in_maps, list(range(NCORES)), trace=_trace)
    out = np.concatenate([res.results[c]["y"] for c in range(NCORES)], axis=0)
    if _trace:
        return out, res
    return out


# revision 24
# speedup vs baseline: 2.4516x; 1.0013x over previous
"""Trainium2 Bass kernel for a LeakyReLU RNN.

Model (B=128, S=512, I=256, H=1024, O=256):
    xproj = lrelu(x @ Wi.T + bi)                          # [B,S,H]
    h_t   = lrelu(concat(xproj_t, h_{t-1}) @ Wh.T + bh)   # recurrence over S
    out   = h_S @ Wo.T + bo                               # [B,O]

Strategy: data-parallel over batch (16 rows/core on 8 cores). Split
Wh = [Wh1 | Wh2]; U = xproj @ Wh1.T + bh is precomputed blockwise into
SBUF, the sequential part is h_t = lrelu(U_t + h_{t-1} @ Wh2.T).

The recurrence matmuls run in bf16 in 128x32 column-tiled PE mode:
three concurrent tiles (TRN2 forbids matmul dst partition 96), each
with its own moving-operand stream, so the per-step Wh2 stream is
~3072 cycles instead of 8192. Tile j covers output features
[384j, min(384j+384, 1024)) at PSUM partitions [32j, 32j+16). Phase-1
GEMM work for block b+2 is interleaved between the recurrence steps of
window b as PE filler, hiding phase 1 and keeping the PE p-state high.
bf16 is safe here: products accumulate in fp32 PSUM, U is injected
from an fp32-accumulated GEMM, and the recurrence is contractive
(||Wh2|| ~ 0.8), so the per-step ~4e-3 quantization error settles
around 1e-2, within the 2e-2 gate.
"""

from contextlib import ExitStack

import ml_dtypes
import numpy as np

import concourse.bacc as bacc
import concourse.tile as tile
from concourse import mybir
from concourse.bass_utils import run_bass_kernel_spmd

B, S, I, H, O = 128, 512, 256, 1024, 256
NCORES = 8
BL = B // NCORES          # batch rows per core = 16
TOK = BL * S              # tokens per core = 8192
ALPHA = 0.01
NSPLIT = (0, 384, 768, 1024)   # 3-way output-feature split for col tiles
# hT slot order: chunks whose transpose source lies in hn cols [0,256)
# first (they unblock after the first activation half), then the rest
KORDER = (0, 1, 3, 4, 6, 7, 2, 5)
import os
ACT_SPLIT = os.environ.get("ACT_SPLIT", "1") == "1"    # lrelu in two halves
COPY_SPLIT = os.environ.get("COPY_SPLIT", "1") == "1"  # psT -> ht in two copies
USE_KORDER = os.environ.get("USE_KORDER", "1") == "1"  # completion-order slots

F32 = mybir.dt.float32
F32R = mybir.dt.float32r
BF16 = mybir.dt.bfloat16
LRELU = mybir.ActivationFunctionType.Lrelu

_CACHED = None


def _build(S=S):
    TOK = BL * S
    NBLK = TOK // 512                 # 512-token (32-step) phase-1 blocks
    nc = bacc.Bacc("TRN2", target_bir_lowering=False, debug=False,
                   num_devices=NCORES)

    xt_d = nc.dram_tensor("xt", [I, TOK], F32, kind="ExternalInput")
    wit_d = nc.dram_tensor("wit", [I, H], F32, kind="ExternalInput")
    wh1t_d = nc.dram_tensor("wh1t", [H, H], F32, kind="ExternalInput")
    wh2b_d = nc.dram_tensor("wh2b", [H, H], BF16, kind="ExternalInput")
    wob_d = nc.dram_tensor("wob", [H, O], BF16, kind="ExternalInput")
    bi_d = nc.dram_tensor("bi", [128, H // 128], F32, kind="ExternalInput")
    bhrep_d = nc.dram_tensor("bhrep", [128, H], F32, kind="ExternalInput")
    borep_d = nc.dram_tensor("borep", [128, O], F32, kind="ExternalInput")
    eyeb_d = nc.dram_tensor("eyeb", [128, 128], BF16, kind="ExternalInput")
    y_d = nc.dram_tensor("y", [BL, O], F32, kind="ExternalOutput")

    with tile.TileContext(nc) as tc, ExitStack() as ctx:
        wpool = ctx.enter_context(tc.tile_pool(name="weights", bufs=1))
        xtpool = ctx.enter_context(tc.tile_pool(name="xt", bufs=2))
        apool = ctx.enter_context(tc.tile_pool(name="atiles", bufs=2))
        upool = ctx.enter_context(tc.tile_pool(name="usb", bufs=2))
        hnpool = ctx.enter_context(tc.tile_pool(name="hn", bufs=2))
        htpool = ctx.enter_context(tc.tile_pool(name="ht", bufs=2))
        opool = ctx.enter_context(tc.tile_pool(name="osb", bufs=1))
        psA = ctx.enter_context(tc.tile_pool(name="psA", bufs=2, space="PSUM"))
        psU = ctx.enter_context(tc.tile_pool(name="psU", bufs=1, space="PSUM"))
        psR = ctx.enter_context(tc.tile_pool(name="psR", bufs=2, space="PSUM"))
        psT = ctx.enter_context(tc.tile_pool(name="psT", bufs=2, space="PSUM"))

        # ---- resident weights ----
        def wload(src, shape, tag, dt=F32R, eng=None):
            t = wpool.tile(shape, dt, tag=tag, name=tag)
            (eng or nc.gpsimd).dma_start(t[:], src)
            return t

        wit = [wload(wit_d.ap()[128 * k:128 * (k + 1), :], [128, H], f"wit{k}")
               for k in range(2)]
        wh1t = [wload(wh1t_d.ap()[128 * k:128 * (k + 1), :], [128, H], f"wh1t{k}")
                for k in range(8)]
        wh2b = [wload(wh2b_d.ap()[128 * k:128 * (k + 1), :], [128, H],
                      f"wh2b{k}", dt=BF16, eng=nc.sync)
                for k in range(8)]
        wob = [wload(wob_d.ap()[128 * k:128 * (k + 1), :], [128, O],
                     f"wob{k}", dt=BF16, eng=nc.sync)
               for k in range(8)]
        eyeb = wload(eyeb_d.ap(), [128, 128], "eyeb", dt=BF16, eng=nc.sync)
        bhrep = wload(bhrep_d.ap(), [128, H], "bhrep", dt=F32, eng=nc.sync)
        borep = wload(borep_d.ap(), [128, O], "borep", dt=F32, eng=nc.sync)
        bi = wpool.tile([128, H // 128], F32, tag="bi", name="bi")
        nc.sync.dma_start(bi[:], bi_d.ap())

        # ---- phase 1 (emitted as filler chunks between recurrence steps)
        # Per 512-token block: A_T = lrelu(WiT.T @ Xt + bi)  (feature-major)
        # then U_g = A_g @ Wh1.T + bh for 4 groups of 128 tokens
        # (token-major [128 tok, 1024] bf16), kept in SBUF for the recurrence.
        usb = {}   # (blk % 2, g) -> SBUF tile [128, H]

        def phase1_chunks(blk, t_base):
            """Return a FIFO of (gate, closure); each closure emits
            ~200-450ns of engine work. A closure may only be emitted at
            absolute step >= gate: the usb evacuations overwrite (pool-wise)
            the buffer the current window's injects still read, so they are
            gated past the last consumer; the psU accumulations for the next
            group must stay behind the previous group's evacuation."""
            c0 = 512 * blk
            chunks = []
            xt = []
            a = []

            def dma_chunk():
                for k in range(2):
                    t = xtpool.tile([128, 512], F32R, tag=f"xt{k}",
                                    name=f"xt{k}_{blk}")
                    nc.gpsimd.dma_start(
                        t[:], xt_d.ap()[128 * k:128 * (k + 1), c0:c0 + 512])
                    xt.append(t)
            chunks.append((0, dma_chunk))

            def a_chunk(m):
                pa = psA.tile([128, 512], F32, tag="psA", name=f"psA_{blk}_{m}")
                nc.tensor.matmul(pa[:], wit[0][:, 128 * m:128 * (m + 1)],
                                 xt[0][:], start=True, stop=False)
                nc.tensor.matmul(pa[:], wit[1][:, 128 * m:128 * (m + 1)],
                                 xt[1][:], start=False, stop=True)
                am = apool.tile([128, 512], F32R, tag=f"a{m}", name=f"a{m}_{blk}")
                nc.scalar.activation(am[:], pa[:], LRELU,
                                     bias=bi[:, m:m + 1], scale=1.0, alpha=ALPHA)
                a.append(am)
            for m in range(8):
                chunks.append((0, lambda m=m: a_chunk(m)))

            # U for group g accumulates into psU [128 tok, 1024]
            # (each matmul dst must stay within one 2KB PSUM bank -> n-halves)
            pu = {}

            def u_chunk(g, k):
                if k == 0:
                    pu[g] = psU.tile([128, H], F32, tag="psU",
                                     name=f"psU_{blk}_{g}")
                for n in range(2):
                    nc.tensor.matmul(pu[g][:, 512 * n:512 * (n + 1)],
                                     a[k][:, 128 * g:128 * (g + 1)],
                                     wh1t[k][:, 512 * n:512 * (n + 1)],
                                     start=(k == 0), stop=(k == 7))

            def u_evac(g, q):
                # evacuate in 256-col slices so the DVE never blocks the
                # recurrence's critical hT copy for long; bf16 cast here
                if q == 0:
                    usb[(blk, g)] = upool.tile([128, H], BF16,
                                               tag=f"usb{g}",
                                               name=f"usb{g}_{blk}")
                sl = slice(256 * q, 256 * (q + 1))
                nc.vector.tensor_add(usb[(blk, g)][:, sl], pu[g][:, sl],
                                     bhrep[:, sl])

            for g in range(4):
                # psU has a single buffer: group g's matmuls may only be
                # emitted once group g-1's evacuation is fully emitted, which
                # the FIFO order plus the evac gates below guarantee.
                for k in range(8):
                    chunks.append((0, lambda g=g, k=k: u_chunk(g, k)))
                for q in range(4):
                    # the usb{g} buffer this evac overwrites is read by the
                    # current window's g-group injects (local steps 8g..8g+7)
                    chunks.append((t_base + 8 * g + 7,
                                   lambda g=g, q=q: u_evac(g, q)))
            return chunks

        # ---- recurrence state ----
        ht = htpool.tile([128, 128], BF16, tag="hT", name="hT_init")
        nc.gpsimd.memset(ht[:].bitcast(F32), 0.0)

        def step(t):
            """One recurrence step: 3 col-tiles of bf16 matmuls + transposes.

            Col tile j covers features [NSPLIT[j], NSPLIT[j+1]) and writes
            PSUM partitions [32j, 32j+16)."""
            nonlocal ht
            blk, g, s = t // 32, (t % 32) // 8, t % 8
            ut = usb[(blk, g)]
            ps = psR.tile([128, 384], F32, tag="psR", name=f"psR_{t}")
            sel = eyeb[:, 16 * s:16 * (s + 1)]
            # inject U_t (start) then accumulate h @ Wh2T, 3 col-tiles
            for j in range(3):
                w = NSPLIT[j + 1] - NSPLIT[j]
                nc.tensor.matmul(ps[32 * j:32 * j + 16, 0:w], sel,
                                 ut[:, NSPLIT[j]:NSPLIT[j + 1]],
                                 start=True, stop=False,
                                 tile_position=(0, 32 * j))
            for o, k in enumerate(KORDER if USE_KORDER else range(8)):
                hk = ht[:, 16 * o:16 * (o + 1)]   # slot o holds chunk k
                for j in range(3):
                    w = NSPLIT[j + 1] - NSPLIT[j]
                    nc.tensor.matmul(ps[32 * j:32 * j + 16, 0:w], hk,
                                     wh2b[k][:, NSPLIT[j]:NSPLIT[j + 1]],
                                     start=False, stop=(o == 7),
                                     tile_position=(0, 32 * j))
            # lrelu split in two column halves so the first six transposes
            # (and the next step's first matmuls) start ~300ns earlier
            hn = hnpool.tile([128, 384], BF16, tag="hn", name=f"hn_{t}")
            if ACT_SPLIT:
                nc.scalar.activation(hn[:, 0:256], ps[:, 0:256], LRELU,
                                     bias=0.0, scale=1.0, alpha=ALPHA)
                nc.scalar.activation(hn[:, 256:384], ps[:, 256:384], LRELU,
                                     bias=0.0, scale=1.0, alpha=ALPHA)
            else:
                nc.scalar.activation(hn[:], ps[:], LRELU,
                                     bias=0.0, scale=1.0, alpha=ALPHA)
            # transpose back to feature-major stationary for the next step,
            # in completion order: chunks fed by the first act half first
            # (psT/ht slot o holds feature chunk KORDER[o])
            pt = psT.tile([128, 128], BF16, tag="psT", name=f"psT_{t}")
            for o, k in enumerate(KORDER if USE_KORDER else range(8)):
                j = (128 * k) // 384          # source col tile
                c = 128 * k - 384 * j         # col offset within tile j
                nc.tensor.transpose(
                    pt[:, 16 * o:16 * (o + 1)],
                    hn[32 * j:32 * j + 16, c:c + 128],
                    eyeb[32 * j:32 * j + 16, 32 * j:32 * j + 16],
                    tile_position=(32 * j, 0))
            ht_new = htpool.tile([128, 128], BF16, tag="hT", name=f"hT_{t}")
            if COPY_SPLIT:
                nc.vector.tensor_copy(ht_new[:, 0:96], pt[:, 0:96])
                nc.vector.tensor_copy(ht_new[:, 96:128], pt[:, 96:128])
            else:
                nc.vector.tensor_copy(ht_new[:], pt[:])
            ht = ht_new

        # ---- schedule: prologue blocks 0,1 then steps with filler ----
        for _, c in phase1_chunks(0, 0):
            c()
        if NBLK > 1:
            for _, c in phase1_chunks(1, 0):
                c()
        queue = []
        for t in range(S):
            if t % 32 == 0:
                nblk = t // 32 + 2
                if nblk < NBLK:
                    queue.extend(phase1_chunks(nblk, t))
            step(t)
            # emit filler chunks to keep the PE busy during the act/copy gap
            budget = 3
            while queue and budget > 0 and queue[0][0] <= t:
                queue.pop(0)[1]()
                budget -= 1

        # ---- phase 3: out = h_S @ Wo.T + bo ----
        po = psR.tile([128, 384], F32, tag="psR", name="psO")
        for o, k in enumerate(KORDER if USE_KORDER else range(8)):
            nc.tensor.matmul(po[0:16, 0:O], ht[:, 16 * o:16 * (o + 1)],
                             wob[k][:], start=(o == 0), stop=(o == 7))
        osb = opool.tile([16, O], F32, tag="osb", name="osb")
        nc.vector.tensor_add(osb[:], po[0:16, 0:O], borep[0:16, :])
        nc.sync.dma_start(y_d.ap(), osb[:])

    nc.compile()
    return nc


def _prep_inputs(x, Wi, bi, Wh, bh, Wo, bo):
    bf = ml_dtypes.bfloat16
    shared = {
        "wit": np.ascontiguousarray(Wi.T),
        "wh1t": np.ascontiguousarray(Wh[:, :H].T),
        "wh2b": np.ascontiguousarray(Wh[:, H:].T).astype(bf),
        "wob": np.ascontiguousarray(Wo.T).astype(bf),
        "bi": np.ascontiguousarray(bi.reshape(H // 128, 128).T),
        "bhrep": np.ascontiguousarray(np.broadcast_to(bh.reshape(1, H),
                                                      (128, H))),
        "borep": np.ascontiguousarray(np.broadcast_to(bo.reshape(1, O),
                                                      (128, O))),
        "eyeb": np.eye(128, dtype=np.float32).astype(bf),
    }
    in_maps = []
    for c in range(NCORES):
        xc = x[BL * c:BL * (c + 1)]            # [16, S, I]
        xt = np.ascontiguousarray(
            xc.transpose(2, 1, 0).reshape(I, xc.shape[1] * BL))
        m = dict(shared)
        m["xt"] = xt
        in_maps.append(m)
    return in_maps


def kernel(x, Wi, bi, Wh, bh, Wo, bo, _trace=False):
    global _CACHED
    x = np.asarray(x, dtype=np.float32)
    if _CACHED is None:
        _CACHED = _build()
    nc = _CACHED
    in_maps = _prep_inputs(np.asarray(x, np.float32), np.asarray(Wi, np.float32),
                           np.asarray(bi, np.float32), np.asarray(Wh, np.float32),
                           np.asarray(bh, np.float32), np.asarray(Wo, np.float32),
                           np.asarray(bo, np.float32))
    res = run_bass_kernel_spmd(nc, in_maps, list(range(NCORES)), trace=_trace)
    out = np.concatenate([res.results[c]["y"] for c in range(NCORES)], axis=0)
    if _trace:
        return out, res
    return out
